# revision 44
# baseline (speedup 1.0000x reference)
"""DenseGATConv Bass/Tile kernel for Trainium2, SPMD over 8 NeuronCores.

Problem (B=4, N=2048, F=128, H=4, C=64):
  xh = (x @ W).reshape(B,N,H,C)
  a_src[b,j,h] = xh . att_src ; a_dst[b,i,h] = xh . att_dst
  s = a_src[j] + a_dst[i];  alpha = softmax_j(mask(adj+I, leaky_relu(s, 0.2)))
  out[b,i] = concat_h(sum_j alpha * xh[b,j,h,:]) + bias

Algebra (no exp over the N*N*H grid, no softmax normalizer subtraction):
  exp(lrelu(s)) / exp(a_dst_i) = max(E1_j * Q'_i, E2_j),
      E1 = exp(0.2 a_src), E2 = exp(a_src), Q' = exp(-0.8 a_dst)
  Masked grid weight  G[j,i] = adjT[j,i] * max(E1_j Q'_i, E2_j).

Work split per (j-tile, head) — all on the DVE by default:
  T = tensor_scalar(Q'_bcast; mult E1, max E2)   (4x mode, 459 ns)
  G = tensor_tensor(T, adjT rep-AP, all 4 heads) (2x mode, 2.2 us)
(ACTN/GPSN env knobs exist to shift T to the Scalar engine - with the
separable E2 branch restored by an extra shared-adj-stationary matmul -
or the TT to GpSimd; both measured net-negative on HW: ACT is 3x slower
per element and strict-FIFO ordering stalls the pipeline, and GpSimd
SBUF-port contention slows concurrent DVE ops ~2.5x.  Default 0/0.)

Accumulation (flipped orientation — no epilogue transposes):
  For each (tile t, i-block ib of 128, head h):
      acc[ib][i, h, c|den] += G_block[j, i]^T @ xh1[j, (c|1)]
  i.e. the 128x128 grid block is the *stationary* operand (FWL-eligible
  fp16 128-col load) and xh1 streams 65 cols.  PSUM acc2[ib] is a full
  bank [128, 4, 128(pad)]; col 64 of each head slot is the softmax
  denominator.  Bias is pre-folded into xh1 (num+bias*den trick), so the
  epilogue is just reciprocal + per-partition tensor_scalar divide + DMA.

Q'_i is broadcast to all 128 partitions with a K=1 ones-stationary
matmul (PSUM bounce) instead of a DRAM roundtrip.

Sharding: core = b*2 + ihalf; each core owns 1024 destination rows of one
batch and reads that batch's full source side (adj slice pre-transposed,
self-loops added, fp16-cast on host; weights pre-folded with the per-head
attention vectors; x / W / projections run in fp16).
"""

import numpy as np

import concourse.bacc as bacc
import concourse.bass as bass
import concourse.tile as tile
from concourse import mybir
from concourse.bass_utils import run_bass_kernel_spmd
from concourse.masks import make_identity

B, N, F = 4, 2048, 128
H, C = 4, 64
HC = H * C
NEG_SLOPE = 0.2
import os
TBUFS = int(os.environ.get('TBUFS', 5))
GBUFS = int(os.environ.get('GBUFS', 6))
ABUFS = int(os.environ.get('ABUFS', 4))
ACTN = int(os.environ.get('ACTN', 0))   # of 16 j-tiles use the ACT path
GPSN = int(os.environ.get('GPSN', 0))   # of 16 j-tiles run their TT on GpSimd
N_CORES = 8
ID = N // 2          # dest rows per core
NT = N // 128        # 16 source tiles
NIB = ID // 128      # 8 dest 128-blocks
F32 = mybir.dt.float32
F16 = mybir.dt.float16

_NC_CACHE = {}


def tile_assignment(actn: int, gpsn: int) -> tuple:
    """Spread actn ACT-path tiles and gpsn gpsimd-TT tiles over the 16
    j-tiles (disjoint sets).  ACT tiles start at t=4 so the Scalar engine
    has drained its startup queue by the time their T grids are needed."""
    actn = max(0, min(actn, NT))
    gpsn = max(0, min(gpsn, NT - actn))
    order = [4, 6, 8, 10, 12, 14, 3, 9, 13, 5, 11, 7, 2, 1, 0, 15]
    acts = set(order[:actn])
    gpss = set(order[actn:actn + gpsn])
    return acts, gpss


def build_nc(reps: int = 1):
    nc = bacc.Bacc("TRN2", target_bir_lowering=False, debug=False, num_devices=1)

    d_xT = nc.dram_tensor("xT", [F, N], F16, kind="ExternalInput").ap()
    d_xTd = nc.dram_tensor("xTd", [F, ID], F16, kind="ExternalInput").ap()
    d_adjT = nc.dram_tensor("adjT", [NT, 128, ID], F16, kind="ExternalInput").ap()
    d_wcat = nc.dram_tensor("Wcat", [F, HC + 8], F16, kind="ExternalInput").ap()
    d_wadst = nc.dram_tensor("Wadst", [F, H], F16, kind="ExternalInput").ap()
    d_bias = nc.dram_tensor("biasv", [1, HC + 8], F16, kind="ExternalInput").ap()
    d_e4 = nc.dram_tensor("e4sel", [4, H * 128], F16, kind="ExternalInput").ap()
    d_out = nc.dram_tensor("out", [ID, HC], F32, kind="ExternalOutput").ap()

    EXP = mybir.ActivationFunctionType.Exp
    CPY = mybir.ActivationFunctionType.Copy
    RELU = mybir.ActivationFunctionType.Relu
    acts, gpss = tile_assignment(ACTN, GPSN)

    with tile.TileContext(nc) as tc:
        with tc.tile_pool(name="const", bufs=1) as const:
            ones1 = const.tile([1, 128], F32)
            nc.vector.memset(ones1, 1.0)
            ones16 = const.tile([1, 128], F16)
            nc.vector.memset(ones16, 1.0)
            z128 = const.tile([1, 128], F16)
            nc.vector.memset(z128, 0.0)
            z512 = const.tile([1, 512], F16)
            nc.vector.memset(z512, 0.0)

            # preload the exp table set while input DMAs run
            scratch1 = const.tile([1, 4], F32)
            nc.scalar.activation(scratch1, ones1[0:1, 0:4], EXP)

            # xTd/wadst first: they gate the q_bc chain that gates the grid
            xTd = const.tile([F, ID], F16)
            nc.sync.dma_start(out=xTd, in_=d_xTd)
            wadst = const.tile([F, H], F16)
            nc.sync.dma_start(out=wadst, in_=d_wadst)
            wcat = const.tile([F, HC + 8], F16)
            nc.sync.dma_start(out=wcat, in_=d_wcat)
            xT = const.tile([F, N], F16)
            for c in range(2):
                nc.sync.dma_start(out=xT[:, c * 1024:(c + 1) * 1024],
                                  in_=d_xT[:, c * 1024:(c + 1) * 1024])
            bias_sb = const.tile([1, HC + 8], F16)
            nc.sync.dma_start(out=bias_sb, in_=d_bias)
            # one-hot selector rows: E4[h] broadcasts qrow4 row h via K=4 MM
            e4 = const.tile([4, H, 128], F16)
            nc.sync.dma_start(out=e4, in_=d_e4)

            # persistent per-core tensors
            xh1 = const.tile([128, NT, H, 65], F16)     # [xh+bias | 1] per (t,h)
            xh2b = const.tile([128, NT, H, 65], F16)    # E2-scaled xh1 (ACT tiles)
            expv = const.tile([128, NT, 8], F32)        # exp(.2 a_src) | exp(a_src)
            nexpv = const.tile([128, NT, 4], F32)       # -exp(a_src) (ACT bias)
            q_bc = const.tile([128, H, ID], F16)        # Q' broadcast per head
            qrow4 = const.tile([4, 2, 512], F16)        # exp(-0.8 a_dst), 4 rows

            tacts = {}
            # ---------------- phase A: projections ----------------
            with tc.tile_pool(name="psA", bufs=2, space="PSUM") as psA, \
                 tc.tile_pool(name="psD", bufs=2, space="PSUM") as psDp, \
                 tc.tile_pool(name="psQ", bufs=2, space="PSUM") as psQp, \
                 tc.tile_pool(name="psB", bufs=2, space="PSUM") as psBp:
                # ones column of every xh1 block (cols 0:64 written below)
                nc.gpsimd.memset(xh1[:, :, :, 64:65], 1.0)
                sc_a = nc.enter_named_scope("phA", False)
                # --- q_bc prefix first: it gates the grid loop.  a_dst
                # projection (all 4 heads in one MM) -> exp -> K=4 one-hot
                # stationary matmul broadcasts row h to all 128 partitions.
                # head 0 is copied PSUM->SBUF on DVE (idle at startup) so the
                # grid pipeline starts asap; the rest go through ACT.
                for k in range(2):
                    psd = psDp.tile([4, 512], F32)
                    nc.tensor.matmul(psd, wadst,
                                     xTd[:, k * 512:(k + 1) * 512],
                                     start=True, stop=True)
                    nc.scalar.activation(qrow4[:, k, :], psd, EXP)
                for h in range(H):
                    for k in range(2):
                        psq = psQp.tile([128, 512], F32)
                        nc.tensor.matmul(psq, e4[:, h, :], qrow4[:, k, :],
                                         start=True, stop=True)
                        if h == 0:
                            # DVE cast: tile 0's first T op (chunk-split)
                            # fires right behind it in the DVE queue
                            nc.vector.tensor_copy(
                                q_bc[:, h, k * 512:(k + 1) * 512], psq)
                        else:
                            nc.scalar.activation(
                                q_bc[:, h, k * 512:(k + 1) * 512], psq, CPY)
                # projection tiles; grid tile t can start once tile t is done
                # (second K=1 matmul accumulates the bias row into the psum,
                # so xh1 = xh + bias with no DVE work — num+bias*den trick)
                for t in range(NT):
                    ps = psA.tile([128, HC + 8], F32)
                    nc.tensor.matmul(ps, xT[:, t * 128:(t + 1) * 128], wcat,
                                     start=True, stop=False)
                    nc.tensor.matmul(ps, ones16, bias_sb,
                                     start=False, stop=True)
                    # exp of the 8 pre-scaled projection cols
                    nc.scalar.activation(expv[:, t, :], ps[:, HC:HC + 8], EXP)
                    # xh+bias into the 65-column head blocks
                    nc.scalar.activation(xh1[:, t, :, 0:64], ps[:, 0:HC], CPY)
                    if t in acts:
                        # negated E2 for the ACT relu bias
                        nc.vector.tensor_scalar(
                            out=nexpv[:, t, :], in0=expv[:, t, 4:8],
                            scalar1=-1.0, scalar2=None,
                            op0=mybir.AluOpType.mult)
                        # E2-scaled stationary for the separable branch
                        for h in range(H):
                            nc.vector.tensor_scalar(
                                out=xh2b[:, t, h, :], in0=xh1[:, t, h, :],
                                scalar1=expv[:, t, 4 + h:5 + h], scalar2=None,
                                op0=mybir.AluOpType.mult)
                        # T = relu(E1_j * Q'_i - E2_j) emitted HERE so the
                        # Scalar engine computes it long before phB's TT
                        # needs it (DVE is strict FIFO — a late T would
                        # head-of-line-block every later DVE op)
                        ta = const.tile([128, H, ID], F16, name=f"tact{t}")
                        for h in range(H):
                            nc.scalar.activation(
                                ta[:, h, :], q_bc[:, h, :], RELU,
                                bias=nexpv[:, t, h:h + 1],
                                scale=expv[:, t, h:h + 1])
                        tacts[t] = ta
                nc.leave_named_scope("phA", sc_a[0], False)

            # ---------------- phase B: grid + matmul accumulate ----------------
            with tc.tile_pool(name="acc", bufs=1, space="PSUM") as accp:
                acc = {}
                for ib in range(NIB):
                    acc[ib] = accp.tile([128, H, 128], F32, tag=f"acc{ib}",
                                        name=f"acc{ib}")

                sc_b = nc.enter_named_scope("phB", False)
                # one whole-bank zeroing matmul per acc bank: carries the only
                # start=True, so per-head accumulate groups sharing a bank
                # can't clear each other's has_written bits
                for ib in range(NIB):
                    accf = bass.AP(
                        tensor=acc[ib].tensor, offset=acc[ib].offset,
                        ap=[acc[ib].ap[0], [1, H * 128]])
                    nc.tensor.matmul(accf, z128, z512, start=True, stop=False)
                with tc.tile_pool(name="adj", bufs=ABUFS) as adjp, \
                     tc.tile_pool(name="grid", bufs=4) as gridp:
                    for rep in range(reps):
                        for t in range(NT):
                            adjt = adjp.tile([128, ID], F16)
                            nc.sync.dma_start(out=adjt, in_=d_adjT[t])
                            edge = (t in (0, NT - 1)
                                    and t not in acts and t not in gpss)
                            if t in acts:
                                t_all = tacts[t]   # precomputed in phA on ACT
                            elif not edge:
                                t_all = gridp.tile([128, H, ID], F16, tag="T",
                                                   bufs=TBUFS)
                                for h in range(H):
                                    # T = max(Q'_i * E1_j, E2_j) on DVE
                                    nc.vector.tensor_scalar(
                                        out=t_all[:, h, :], in0=q_bc[:, h, :],
                                        scalar1=expv[:, t, h:h + 1],
                                        scalar2=expv[:, t, 4 + h:5 + h],
                                        op0=mybir.AluOpType.mult,
                                        op1=mybir.AluOpType.max)
                            g = gridp.tile([128, H, ID], F16, tag="G", bufs=GBUFS)
                            if edge:
                                # first/last tile: chunk-split T+TT so the
                                # first MMs start (t=0) / the epilogue's MMs
                                # finish (t=15) half a grid earlier
                                t_all = gridp.tile([128, H, ID], F16, tag="T",
                                                   bufs=TBUFS)
                                for k2 in range(2):
                                    sl = slice(k2 * 512, (k2 + 1) * 512)
                                    for h in range(H):
                                        nc.vector.tensor_scalar(
                                            out=t_all[:, h, sl],
                                            in0=q_bc[:, h, sl],
                                            scalar1=expv[:, t, h:h + 1],
                                            scalar2=expv[:, t, 4 + h:5 + h],
                                            op0=mybir.AluOpType.mult,
                                            op1=mybir.AluOpType.max)
                                    tv = bass.AP(
                                        tensor=t_all.tensor,
                                        offset=t_all.offset + k2 * 512,
                                        ap=[t_all.ap[0], [ID, H], [1, 512]])
                                    gv = bass.AP(
                                        tensor=g.tensor,
                                        offset=g.offset + k2 * 512,
                                        ap=[g.ap[0], [ID, H], [1, 512]])
                                    adj_rep4k = bass.AP(
                                        tensor=adjt.tensor,
                                        offset=adjt.offset + k2 * 512,
                                        ap=[adjt.ap[0], [0, H], [1, 512]])
                                    nc.vector.tensor_tensor(
                                        out=gv, in0=tv, in1=adj_rep4k,
                                        op=mybir.AluOpType.mult)
                            elif t in gpss:
                                # gpsimd path: per-head TT (natural in1 AP)
                                for h in range(H):
                                    nc.gpsimd.tensor_tensor(
                                        out=g[:, h, :], in0=t_all[:, h, :],
                                        in1=adjt, op=mybir.AluOpType.mult)
                            else:
                                adj_rep4 = bass.AP(
                                    tensor=adjt.tensor, offset=adjt.offset,
                                    ap=[adjt.ap[0], [0, H]] + list(adjt.ap[1:]))
                                nc.vector.tensor_tensor(
                                    out=g, in0=t_all, in1=adj_rep4,
                                    op=mybir.AluOpType.mult)
                            last = (rep == reps - 1 and t == NT - 1)
                            for ib in range(NIB):
                                for h in range(H):
                                    # acc[ib][i, h, :] += G_blk^T @ [xh|1]
                                    nc.tensor.matmul(
                                        acc[ib][:, h, 0:65],
                                        g[:, h, ib * 128:(ib + 1) * 128],
                                        xh1[:, t, h, :],
                                        start=False, stop=last)
                                if t in acts:
                                    # separable E2 branch: shared adj block
                                    # stationary, all 4 heads' E2-xh moving
                                    nc.tensor.matmul(
                                        acc[ib][:, :, 0:65],
                                        adjt[:, ib * 128:(ib + 1) * 128],
                                        xh2b[:, t, :, :],
                                        start=False, stop=False)

                nc.leave_named_scope("phB", sc_b[0], False)
                sc_c = nc.enter_named_scope("phC", False)
                # ------------- phase C: divide + out (no transposes) -------------
                with tc.tile_pool(name="ep_sm", bufs=8) as epsm, \
                     tc.tile_pool(name="outp", bufs=2) as outp:
                    # 4 dest blocks share one osb group so every partition
                    # DMAs 4KB contiguous (host un-permutes the row order)
                    for g in range(NIB // 4):
                        osb = outp.tile([128, 4, HC], F32, tag="osb",
                                        name="osb")
                        for kb in range(4):
                            ib = g * 4 + kb
                            rec = epsm.tile([128, H, 1], F32)
                            nc.vector.reciprocal(rec, acc[ib][:, :, 64:65])
                            for h in range(H):
                                if h % 2 == 0:
                                    nc.vector.tensor_scalar(
                                        out=osb[:, kb, h * 64:(h + 1) * 64],
                                        in0=acc[ib][:, h, 0:64],
                                        scalar1=rec[:, h, :], scalar2=None,
                                        op0=mybir.AluOpType.mult)
                                else:
                                    nc.scalar.activation(
                                        osb[:, kb, h * 64:(h + 1) * 64],
                                        acc[ib][:, h, 0:64], CPY,
                                        scale=rec[:, h, :])
                        blk = d_out[g * 512:(g + 1) * 512, :]
                        out_ap = bass.AP(
                            tensor=blk.tensor, offset=blk.offset,
                            ap=[[4 * HC, 128], [HC, 4], [1, HC]])
                        nc.sync.dma_start(out=out_ap, in_=osb)
                nc.leave_named_scope("phC", sc_c[0], False)

    nc.compile()
    return nc


def _get_nc(reps: int = 1):
    if reps not in _NC_CACHE:
        _NC_CACHE[reps] = build_nc(reps)
    return _NC_CACHE[reps]


def make_in_maps(x, adj, W, att_src, att_dst, bias):
    x = np.asarray(x, dtype=np.float32)
    adj = np.asarray(adj, dtype=np.float32)
    W = np.asarray(W, dtype=np.float32)
    att_src = np.asarray(att_src, dtype=np.float32)
    att_dst = np.asarray(att_dst, dtype=np.float32)
    bias = np.asarray(bias, dtype=np.float32)

    # weight prep: fold per-head attention dots into projection columns
    wa_src = np.stack([W[:, h * C:(h + 1) * C] @ att_src[h] for h in range(H)], 1)
    wa_dst = np.stack([W[:, h * C:(h + 1) * C] @ att_dst[h] for h in range(H)], 1)
    wcat = np.concatenate([W, 0.2 * wa_src, wa_src], axis=1)
    wcat = np.ascontiguousarray(wcat, dtype=np.float16)          # [F, 264]
    wadst = np.ascontiguousarray(-0.8 * wa_dst, dtype=np.float16)  # [F, 4]

    adjl = adj.copy()
    idx = np.arange(N)
    adjl[:, idx, idx] = 1.0

    # one-hot broadcast selector: e4sel[p, h*128+c] = (p == h)
    e4sel = np.zeros((4, H, 128), np.float16)
    for h in range(H):
        e4sel[h, h, :] = 1.0
    e4sel = e4sel.reshape(4, H * 128)

    in_maps = []
    for c in range(N_CORES):
        b, half = c // 2, c % 2
        xT = np.ascontiguousarray(x[b].T, dtype=np.float16)
        xTd = np.ascontiguousarray(x[b, half * ID:(half + 1) * ID, :].T,
                                   dtype=np.float16)
        adjT = np.ascontiguousarray(
            adjl[b].T[:, half * ID:(half + 1) * ID]).astype(np.float16)
        in_maps.append({
            "xT": xT,
            "xTd": xTd,
            "adjT": adjT.reshape(NT, 128, ID),
            "Wcat": wcat,
            "Wadst": wadst,
            "biasv": np.concatenate([bias, np.zeros(8, np.float32)]).reshape(
                1, HC + 8).astype(np.float16),
            "e4sel": e4sel,
        })
    return in_maps


def assemble(results):
    # un-permute the 4-blocks-per-DMA row grouping: DRAM row
    # (ib//4)*512 + 4*p + ib%4 holds original row ib*128 + p
    i = np.arange(ID)
    ib, p = i // 128, i % 128
    r = (ib // 4) * 512 + 4 * p + (ib % 4)
    out = np.empty((B, N, HC), dtype=np.float32)
    for c in range(N_CORES):
        b, half = c // 2, c % 2
        out[b, half * ID:(half + 1) * ID, :] = results[c]["out"][r]
    return out


def kernel(x, adj, W, att_src, att_dst, bias):
    nc = _get_nc(1)
    in_maps = make_in_maps(x, adj, W, att_src, att_dst, bias)
    res = run_bass_kernel_spmd(nc, in_maps, list(range(N_CORES)))
    return assemble(res.results)


# revision 45
# speedup vs baseline: 1.0136x; 1.0136x over previous
"""DenseGATConv Bass/Tile kernel for Trainium2, SPMD over 8 NeuronCores.

Problem (B=4, N=2048, F=128, H=4, C=64):
  xh = (x @ W).reshape(B,N,H,C)
  a_src[b,j,h] = xh . att_src ; a_dst[b,i,h] = xh . att_dst
  s = a_src[j] + a_dst[i];  alpha = softmax_j(mask(adj+I, leaky_relu(s, 0.2)))
  out[b,i] = concat_h(sum_j alpha * xh[b,j,h,:]) + bias

Algebra (no exp over the N*N*H grid, no softmax normalizer subtraction):
  exp(lrelu(s)) / exp(a_dst_i) = max(E1_j * Q'_i, E2_j),
      E1 = exp(0.2 a_src), E2 = exp(a_src), Q' = exp(-0.8 a_dst)
  Masked grid weight  G[j,i] = adjT[j,i] * max(E1_j Q'_i, E2_j).

Work split per (j-tile, head) — all on the DVE by default:
  T = tensor_scalar(Q'_bcast; mult E1, max E2)   (4x mode, 459 ns)
  G = tensor_tensor(T, adjT rep-AP, all 4 heads) (2x mode, 2.2 us)
(ACTN/GPSN env knobs exist to shift T to the Scalar engine - with the
separable E2 branch restored by an extra shared-adj-stationary matmul -
or the TT to GpSimd; both measured net-negative on HW: ACT is 3x slower
per element and strict-FIFO ordering stalls the pipeline, and GpSimd
SBUF-port contention slows concurrent DVE ops ~2.5x.  Default 0/0.)

Accumulation (flipped orientation — no epilogue transposes):
  For each (tile t, i-block ib of 128, head h):
      acc[ib][i, h, c|den] += G_block[j, i]^T @ xh1[j, (c|1)]
  i.e. the 128x128 grid block is the *stationary* operand (FWL-eligible
  fp16 128-col load) and xh1 streams 65 cols.  PSUM acc2[ib] is a full
  bank [128, 4, 128(pad)]; col 64 of each head slot is the softmax
  denominator.  Bias is pre-folded into xh1 (num+bias*den trick), so the
  epilogue is just reciprocal + per-partition tensor_scalar divide + DMA.

Q'_i is broadcast to all 128 partitions with a K=1 ones-stationary
matmul (PSUM bounce) instead of a DRAM roundtrip.

Sharding: core = b*2 + ihalf; each core owns 1024 destination rows of one
batch and reads that batch's full source side (adj slice pre-transposed,
self-loops added, fp16-cast on host; weights pre-folded with the per-head
attention vectors; x / W / projections run in fp16).
"""

import numpy as np

import concourse.bacc as bacc
import concourse.bass as bass
import concourse.tile as tile
from concourse import mybir
from concourse.bass_utils import run_bass_kernel_spmd
from concourse.masks import make_identity

B, N, F = 4, 2048, 128
H, C = 4, 64
HC = H * C
NEG_SLOPE = 0.2
import os
TBUFS = int(os.environ.get('TBUFS', 5))
GBUFS = int(os.environ.get('GBUFS', 6))
ABUFS = int(os.environ.get('ABUFS', 4))
ACTN = int(os.environ.get('ACTN', 0))   # of 16 j-tiles use the ACT path
GPSN = int(os.environ.get('GPSN', 0))   # of 16 j-tiles run their TT on GpSimd
N_CORES = 8
ID = N // 2          # dest rows per core
NT = N // 128        # 16 source tiles
NIB = ID // 128      # 8 dest 128-blocks
F32 = mybir.dt.float32
F16 = mybir.dt.float16

_NC_CACHE = {}


def tile_assignment(actn: int, gpsn: int) -> tuple:
    """Spread actn ACT-path tiles and gpsn gpsimd-TT tiles over the 16
    j-tiles (disjoint sets).  ACT tiles start at t=4 so the Scalar engine
    has drained its startup queue by the time their T grids are needed."""
    actn = max(0, min(actn, NT))
    gpsn = max(0, min(gpsn, NT - actn))
    order = [4, 6, 8, 10, 12, 14, 3, 9, 13, 5, 11, 7, 2, 1, 0, 15]
    acts = set(order[:actn])
    gpss = set(order[actn:actn + gpsn])
    return acts, gpss


def build_nc(reps: int = 1):
    nc = bacc.Bacc("TRN2", target_bir_lowering=False, debug=False, num_devices=1)

    d_xT = nc.dram_tensor("xT", [F, N], F16, kind="ExternalInput").ap()
    d_xTd = nc.dram_tensor("xTd", [F, ID], F16, kind="ExternalInput").ap()
    d_adjT = nc.dram_tensor("adjT", [NT, 128, ID], F16, kind="ExternalInput").ap()
    d_wcat = nc.dram_tensor("Wcat", [F, HC + 8], F16, kind="ExternalInput").ap()
    d_wadst = nc.dram_tensor("Wadst", [F, H], F16, kind="ExternalInput").ap()
    d_bias = nc.dram_tensor("biasv", [1, HC + 8], F16, kind="ExternalInput").ap()
    d_e4 = nc.dram_tensor("e4sel", [4, H * 128], F16, kind="ExternalInput").ap()
    d_out = nc.dram_tensor("out", [ID, HC], F32, kind="ExternalOutput").ap()

    EXP = mybir.ActivationFunctionType.Exp
    CPY = mybir.ActivationFunctionType.Copy
    RELU = mybir.ActivationFunctionType.Relu
    acts, gpss = tile_assignment(ACTN, GPSN)

    with tile.TileContext(nc) as tc:
        with tc.tile_pool(name="const", bufs=1) as const:
            ones1 = const.tile([1, 128], F32)
            nc.vector.memset(ones1, 1.0)
            ones16 = const.tile([1, 128], F16)
            nc.vector.memset(ones16, 1.0)
            z128 = const.tile([1, 128], F16)
            nc.vector.memset(z128, 0.0)
            z512 = const.tile([1, 512], F16)
            nc.vector.memset(z512, 0.0)

            # preload the exp table set while input DMAs run
            scratch1 = const.tile([1, 4], F32)
            nc.scalar.activation(scratch1, ones1[0:1, 0:4], EXP)

            # xTd/wadst first: they gate the q_bc chain that gates the grid
            xTd = const.tile([F, ID], F16)
            nc.sync.dma_start(out=xTd, in_=d_xTd)
            wadst = const.tile([F, H], F16)
            nc.sync.dma_start(out=wadst, in_=d_wadst)
            wcat = const.tile([F, HC + 8], F16)
            nc.sync.dma_start(out=wcat, in_=d_wcat)
            xT = const.tile([F, N], F16)
            for c in range(2):
                nc.sync.dma_start(out=xT[:, c * 1024:(c + 1) * 1024],
                                  in_=d_xT[:, c * 1024:(c + 1) * 1024])
            bias_sb = const.tile([1, HC + 8], F16)
            nc.sync.dma_start(out=bias_sb, in_=d_bias)
            # one-hot selector rows: E4[h] broadcasts qrow4 row h via K=4 MM
            e4 = const.tile([4, H, 128], F16)
            nc.sync.dma_start(out=e4, in_=d_e4)

            # persistent per-core tensors
            xh1 = const.tile([128, NT, H, 65], F16)     # [xh+bias | 1] per (t,h)
            xh2b = const.tile([128, NT, H, 65], F16)    # E2-scaled xh1 (ACT tiles)
            expv = const.tile([128, NT, 8], F32)        # exp(.2 a_src) | exp(a_src)
            nexpv = const.tile([128, NT, 4], F32)       # -exp(a_src) (ACT bias)
            q_bc = const.tile([128, H, ID], F16)        # Q' broadcast per head
            qrow4 = const.tile([4, 2, 512], F16)        # exp(-0.8 a_dst), 4 rows

            tacts = {}
            # ---------------- phase A: projections ----------------
            with tc.tile_pool(name="psA", bufs=2, space="PSUM") as psA, \
                 tc.tile_pool(name="psD", bufs=2, space="PSUM") as psDp, \
                 tc.tile_pool(name="psQ", bufs=2, space="PSUM") as psQp, \
                 tc.tile_pool(name="psB", bufs=2, space="PSUM") as psBp:
                # ones column of every xh1 block (cols 0:64 written below)
                nc.gpsimd.memset(xh1[:, :, :, 64:65], 1.0)
                sc_a = nc.enter_named_scope("phA", False)
                # --- q_bc prefix first: it gates the grid loop.  a_dst
                # projection (all 4 heads in one MM) -> exp -> K=4 one-hot
                # stationary matmul broadcasts row h to all 128 partitions.
                # head 0 is copied PSUM->SBUF on DVE (idle at startup) so the
                # grid pipeline starts asap; the rest go through ACT.
                for k in range(2):
                    psd = psDp.tile([4, 512], F32)
                    nc.tensor.matmul(psd, wadst,
                                     xTd[:, k * 512:(k + 1) * 512],
                                     start=True, stop=True)
                    nc.scalar.activation(qrow4[:, k, :], psd, EXP)
                for h in range(H):
                    for k in range(2):
                        psq = psQp.tile([128, 512], F32)
                        nc.tensor.matmul(psq, e4[:, h, :], qrow4[:, k, :],
                                         start=True, stop=True)
                        nc.scalar.activation(
                            q_bc[:, h, k * 512:(k + 1) * 512], psq, CPY)
                # projection tiles; grid tile t can start once tile t is done
                # (second K=1 matmul accumulates the bias row into the psum,
                # so xh1 = xh + bias with no DVE work — num+bias*den trick)
                for t in range(NT):
                    ps = psA.tile([128, HC + 8], F32)
                    nc.tensor.matmul(ps, xT[:, t * 128:(t + 1) * 128], wcat,
                                     start=True, stop=False)
                    nc.tensor.matmul(ps, ones16, bias_sb,
                                     start=False, stop=True)
                    # exp of the 8 pre-scaled projection cols
                    nc.scalar.activation(expv[:, t, :], ps[:, HC:HC + 8], EXP)
                    # xh+bias into the 65-column head blocks
                    nc.scalar.activation(xh1[:, t, :, 0:64], ps[:, 0:HC], CPY)
                    if t in acts:
                        # negated E2 for the ACT relu bias
                        nc.vector.tensor_scalar(
                            out=nexpv[:, t, :], in0=expv[:, t, 4:8],
                            scalar1=-1.0, scalar2=None,
                            op0=mybir.AluOpType.mult)
                        # E2-scaled stationary for the separable branch
                        for h in range(H):
                            nc.vector.tensor_scalar(
                                out=xh2b[:, t, h, :], in0=xh1[:, t, h, :],
                                scalar1=expv[:, t, 4 + h:5 + h], scalar2=None,
                                op0=mybir.AluOpType.mult)
                        # T = relu(E1_j * Q'_i - E2_j) emitted HERE so the
                        # Scalar engine computes it long before phB's TT
                        # needs it (DVE is strict FIFO — a late T would
                        # head-of-line-block every later DVE op)
                        ta = const.tile([128, H, ID], F16, name=f"tact{t}")
                        for h in range(H):
                            nc.scalar.activation(
                                ta[:, h, :], q_bc[:, h, :], RELU,
                                bias=nexpv[:, t, h:h + 1],
                                scale=expv[:, t, h:h + 1])
                        tacts[t] = ta
                nc.leave_named_scope("phA", sc_a[0], False)

            # ---------------- phase B: grid + matmul accumulate ----------------
            with tc.tile_pool(name="acc", bufs=1, space="PSUM") as accp:
                acc = {}
                for ib in range(NIB):
                    acc[ib] = accp.tile([128, H, 128], F32, tag=f"acc{ib}",
                                        name=f"acc{ib}")

                sc_b = nc.enter_named_scope("phB", False)
                # one whole-bank zeroing matmul per acc bank: carries the only
                # start=True, so per-head accumulate groups sharing a bank
                # can't clear each other's has_written bits
                for ib in range(NIB):
                    accf = bass.AP(
                        tensor=acc[ib].tensor, offset=acc[ib].offset,
                        ap=[acc[ib].ap[0], [1, H * 128]])
                    nc.tensor.matmul(accf, z128, z512, start=True, stop=False)
                with tc.tile_pool(name="adj", bufs=ABUFS) as adjp, \
                     tc.tile_pool(name="grid", bufs=4) as gridp:
                    for rep in range(reps):
                        for t in range(NT):
                            adjt = adjp.tile([128, ID], F16)
                            nc.sync.dma_start(out=adjt, in_=d_adjT[t])
                            edge = (t in (0, NT - 1)
                                    and t not in acts and t not in gpss)
                            if t in acts:
                                t_all = tacts[t]   # precomputed in phA on ACT
                            elif not edge:
                                t_all = gridp.tile([128, H, ID], F16, tag="T",
                                                   bufs=TBUFS)
                                for h in range(H):
                                    # T = max(Q'_i * E1_j, E2_j) on DVE
                                    nc.vector.tensor_scalar(
                                        out=t_all[:, h, :], in0=q_bc[:, h, :],
                                        scalar1=expv[:, t, h:h + 1],
                                        scalar2=expv[:, t, 4 + h:5 + h],
                                        op0=mybir.AluOpType.mult,
                                        op1=mybir.AluOpType.max)
                            g = gridp.tile([128, H, ID], F16, tag="G", bufs=GBUFS)
                            if edge:
                                # first/last tile: chunk-split T+TT so the
                                # first MMs start (t=0) / the epilogue's MMs
                                # finish (t=15) half a grid earlier
                                t_all = gridp.tile([128, H, ID], F16, tag="T",
                                                   bufs=TBUFS)
                                for k2 in range(2):
                                    sl = slice(k2 * 512, (k2 + 1) * 512)
                                    for h in range(H):
                                        nc.vector.tensor_scalar(
                                            out=t_all[:, h, sl],
                                            in0=q_bc[:, h, sl],
                                            scalar1=expv[:, t, h:h + 1],
                                            scalar2=expv[:, t, 4 + h:5 + h],
                                            op0=mybir.AluOpType.mult,
                                            op1=mybir.AluOpType.max)
                                    tv = bass.AP(
                                        tensor=t_all.tensor,
                                        offset=t_all.offset + k2 * 512,
                                        ap=[t_all.ap[0], [ID, H], [1, 512]])
                                    gv = bass.AP(
                                        tensor=g.tensor,
                                        offset=g.offset + k2 * 512,
                                        ap=[g.ap[0], [ID, H], [1, 512]])
                                    adj_rep4k = bass.AP(
                                        tensor=adjt.tensor,
                                        offset=adjt.offset + k2 * 512,
                                        ap=[adjt.ap[0], [0, H], [1, 512]])
                                    nc.vector.tensor_tensor(
                                        out=gv, in0=tv, in1=adj_rep4k,
                                        op=mybir.AluOpType.mult)
                            elif t in gpss:
                                # gpsimd path: per-head TT (natural in1 AP)
                                for h in range(H):
                                    nc.gpsimd.tensor_tensor(
                                        out=g[:, h, :], in0=t_all[:, h, :],
                                        in1=adjt, op=mybir.AluOpType.mult)
                            else:
                                adj_rep4 = bass.AP(
                                    tensor=adjt.tensor, offset=adjt.offset,
                                    ap=[adjt.ap[0], [0, H]] + list(adjt.ap[1:]))
                                nc.vector.tensor_tensor(
                                    out=g, in0=t_all, in1=adj_rep4,
                                    op=mybir.AluOpType.mult)
                            last = (rep == reps - 1 and t == NT - 1)
                            for ib in range(NIB):
                                for h in range(H):
                                    # acc[ib][i, h, :] += G_blk^T @ [xh|1]
                                    nc.tensor.matmul(
                                        acc[ib][:, h, 0:65],
                                        g[:, h, ib * 128:(ib + 1) * 128],
                                        xh1[:, t, h, :],
                                        start=False, stop=last)
                                if t in acts:
                                    # separable E2 branch: shared adj block
                                    # stationary, all 4 heads' E2-xh moving
                                    nc.tensor.matmul(
                                        acc[ib][:, :, 0:65],
                                        adjt[:, ib * 128:(ib + 1) * 128],
                                        xh2b[:, t, :, :],
                                        start=False, stop=False)

                nc.leave_named_scope("phB", sc_b[0], False)
                sc_c = nc.enter_named_scope("phC", False)
                # ------------- phase C: divide + out (no transposes) -------------
                with tc.tile_pool(name="ep_sm", bufs=8) as epsm, \
                     tc.tile_pool(name="outp", bufs=2) as outp:
                    # 4 dest blocks share one osb group so every partition
                    # DMAs 4KB contiguous (host un-permutes the row order)
                    for g in range(NIB // 4):
                        osb = outp.tile([128, 4, HC], F32, tag="osb",
                                        name="osb")
                        for kb in range(4):
                            ib = g * 4 + kb
                            rec = epsm.tile([128, H, 1], F32)
                            nc.vector.reciprocal(rec, acc[ib][:, :, 64:65])
                            for h in range(H):
                                if h % 2 == 0:
                                    nc.vector.tensor_scalar(
                                        out=osb[:, kb, h * 64:(h + 1) * 64],
                                        in0=acc[ib][:, h, 0:64],
                                        scalar1=rec[:, h, :], scalar2=None,
                                        op0=mybir.AluOpType.mult)
                                else:
                                    nc.scalar.activation(
                                        osb[:, kb, h * 64:(h + 1) * 64],
                                        acc[ib][:, h, 0:64], CPY,
                                        scale=rec[:, h, :])
                        blk = d_out[g * 512:(g + 1) * 512, :]
                        out_ap = bass.AP(
                            tensor=blk.tensor, offset=blk.offset,
                            ap=[[4 * HC, 128], [HC, 4], [1, HC]])
                        nc.sync.dma_start(out=out_ap, in_=osb)
                nc.leave_named_scope("phC", sc_c[0], False)

    nc.compile()
    return nc


def _get_nc(reps: int = 1):
    if reps not in _NC_CACHE:
        _NC_CACHE[reps] = build_nc(reps)
    return _NC_CACHE[reps]


def make_in_maps(x, adj, W, att_src, att_dst, bias):
    x = np.asarray(x, dtype=np.float32)
    adj = np.asarray(adj, dtype=np.float32)
    W = np.asarray(W, dtype=np.float32)
    att_src = np.asarray(att_src, dtype=np.float32)
    att_dst = np.asarray(att_dst, dtype=np.float32)
    bias = np.asarray(bias, dtype=np.float32)

    # weight prep: fold per-head attention dots into projection columns
    wa_src = np.stack([W[:, h * C:(h + 1) * C] @ att_src[h] for h in range(H)], 1)
    wa_dst = np.stack([W[:, h * C:(h + 1) * C] @ att_dst[h] for h in range(H)], 1)
    wcat = np.concatenate([W, 0.2 * wa_src, wa_src], axis=1)
    wcat = np.ascontiguousarray(wcat, dtype=np.float16)          # [F, 264]
    wadst = np.ascontiguousarray(-0.8 * wa_dst, dtype=np.float16)  # [F, 4]

    adjl = adj.copy()
    idx = np.arange(N)
    adjl[:, idx, idx] = 1.0

    # one-hot broadcast selector: e4sel[p, h*128+c] = (p == h)
    e4sel = np.zeros((4, H, 128), np.float16)
    for h in range(H):
        e4sel[h, h, :] = 1.0
    e4sel = e4sel.reshape(4, H * 128)

    in_maps = []
    for c in range(N_CORES):
        b, half = c // 2, c % 2
        xT = np.ascontiguousarray(x[b].T, dtype=np.float16)
        xTd = np.ascontiguousarray(x[b, half * ID:(half + 1) * ID, :].T,
                                   dtype=np.float16)
        adjT = np.ascontiguousarray(
            adjl[b].T[:, half * ID:(half + 1) * ID]).astype(np.float16)
        in_maps.append({
            "xT": xT,
            "xTd": xTd,
            "adjT": adjT.reshape(NT, 128, ID),
            "Wcat": wcat,
            "Wadst": wadst,
            "biasv": np.concatenate([bias, np.zeros(8, np.float32)]).reshape(
                1, HC + 8).astype(np.float16),
            "e4sel": e4sel,
        })
    return in_maps


def assemble(results):
    # un-permute the 4-blocks-per-DMA row grouping: DRAM row
    # (ib//4)*512 + 4*p + ib%4 holds original row ib*128 + p
    i = np.arange(ID)
    ib, p = i // 128, i % 128
    r = (ib // 4) * 512 + 4 * p + (ib % 4)
    out = np.empty((B, N, HC), dtype=np.float32)
    for c in range(N_CORES):
        b, half = c // 2, c % 2
        out[b, half * ID:(half + 1) * ID, :] = results[c]["out"][r]
    return out


def kernel(x, adj, W, att_src, att_dst, bias):
    nc = _get_nc(1)
    in_maps = make_in_maps(x, adj, W, att_src, att_dst, bias)
    res = run_bass_kernel_spmd(nc, in_maps, list(range(N_CORES)))
    return assemble(res.results)


# revision 46
# speedup vs baseline: 1.0136x; 1.0001x over previous
"""DenseGATConv Bass/Tile kernel for Trainium2, SPMD over 8 NeuronCores.

Problem (B=4, N=2048, F=128, H=4, C=64):
  xh = (x @ W).reshape(B,N,H,C)
  a_src[b,j,h] = xh . att_src ; a_dst[b,i,h] = xh . att_dst
  s = a_src[j] + a_dst[i];  alpha = softmax_j(mask(adj+I, leaky_relu(s, 0.2)))
  out[b,i] = concat_h(sum_j alpha * xh[b,j,h,:]) + bias

Algebra (no exp over the N*N*H grid, no softmax normalizer subtraction):
  exp(lrelu(s)) / exp(a_dst_i) = max(E1_j * Q'_i, E2_j),
      E1 = exp(0.2 a_src), E2 = exp(a_src), Q' = exp(-0.8 a_dst)
  Masked grid weight  G[j,i] = adjT[j,i] * max(E1_j Q'_i, E2_j).

Work split per (j-tile, head) — all on the DVE by default:
  T = tensor_scalar(Q'_bcast; mult E1, max E2)   (4x mode, 459 ns)
  G = tensor_tensor(T, adjT rep-AP, all 4 heads) (2x mode, 2.2 us)
(ACTN/GPSN env knobs exist to shift T to the Scalar engine - with the
separable E2 branch restored by an extra shared-adj-stationary matmul -
or the TT to GpSimd; both measured net-negative on HW: ACT is 3x slower
per element and strict-FIFO ordering stalls the pipeline, and GpSimd
SBUF-port contention slows concurrent DVE ops ~2.5x.  Default 0/0.)

Accumulation (flipped orientation — no epilogue transposes):
  For each (tile t, i-block ib of 128, head h):
      acc[ib][i, h, c|den] += G_block[j, i]^T @ xh1[j, (c|1)]
  i.e. the 128x128 grid block is the *stationary* operand (FWL-eligible
  fp16 128-col load) and xh1 streams 65 cols.  PSUM acc2[ib] is a full
  bank [128, 4, 128(pad)]; col 64 of each head slot is the softmax
  denominator.  Bias is pre-folded into xh1 (num+bias*den trick), so the
  epilogue is just reciprocal + per-partition tensor_scalar divide + DMA.

Q'_i is broadcast to all 128 partitions with a K=1 ones-stationary
matmul (PSUM bounce) instead of a DRAM roundtrip.

Sharding: core = b*2 + ihalf; each core owns 1024 destination rows of one
batch and reads that batch's full source side (adj slice pre-transposed,
self-loops added, fp16-cast on host; weights pre-folded with the per-head
attention vectors; x / W / projections run in fp16).
"""

import numpy as np

import concourse.bacc as bacc
import concourse.bass as bass
import concourse.tile as tile
from concourse import mybir
from concourse.bass_utils import run_bass_kernel_spmd
from concourse.masks import make_identity

B, N, F = 4, 2048, 128
H, C = 4, 64
HC = H * C
NEG_SLOPE = 0.2
import os
TBUFS = int(os.environ.get('TBUFS', 5))
GBUFS = int(os.environ.get('GBUFS', 6))
ABUFS = int(os.environ.get('ABUFS', 4))
ACTN = int(os.environ.get('ACTN', 0))   # of 16 j-tiles use the ACT path
GPSN = int(os.environ.get('GPSN', 0))   # of 16 j-tiles run their TT on GpSimd
N_CORES = 8
ID = N // 2          # dest rows per core
NT = N // 128        # 16 source tiles
NIB = ID // 128      # 8 dest 128-blocks
F32 = mybir.dt.float32
F16 = mybir.dt.float16

_NC_CACHE = {}


def tile_assignment(actn: int, gpsn: int) -> tuple:
    """Spread actn ACT-path tiles and gpsn gpsimd-TT tiles over the 16
    j-tiles (disjoint sets).  ACT tiles start at t=4 so the Scalar engine
    has drained its startup queue by the time their T grids are needed."""
    actn = max(0, min(actn, NT))
    gpsn = max(0, min(gpsn, NT - actn))
    order = [4, 6, 8, 10, 12, 14, 3, 9, 13, 5, 11, 7, 2, 1, 0, 15]
    acts = set(order[:actn])
    gpss = set(order[actn:actn + gpsn])
    return acts, gpss


def build_nc(reps: int = 1):
    nc = bacc.Bacc("TRN2", target_bir_lowering=False, debug=False, num_devices=1)

    d_xT = nc.dram_tensor("xT", [F, N], F16, kind="ExternalInput").ap()
    d_xTd = nc.dram_tensor("xTd", [F, ID], F16, kind="ExternalInput").ap()
    d_adjT = nc.dram_tensor("adjT", [NT, 128, ID], F16, kind="ExternalInput").ap()
    d_wcat = nc.dram_tensor("Wcat", [F, HC + 8], F16, kind="ExternalInput").ap()
    d_wadst = nc.dram_tensor("Wadst", [F, H], F16, kind="ExternalInput").ap()
    d_bias = nc.dram_tensor("biasv", [1, HC + 8], F16, kind="ExternalInput").ap()
    d_e4 = nc.dram_tensor("e4sel", [4, H * 128], F16, kind="ExternalInput").ap()
    d_out = nc.dram_tensor("out", [ID, HC], F32, kind="ExternalOutput").ap()

    EXP = mybir.ActivationFunctionType.Exp
    CPY = mybir.ActivationFunctionType.Copy
    RELU = mybir.ActivationFunctionType.Relu
    acts, gpss = tile_assignment(ACTN, GPSN)

    with tile.TileContext(nc) as tc:
        with tc.tile_pool(name="const", bufs=1) as const:
            ones1 = const.tile([1, 128], F32)
            nc.vector.memset(ones1, 1.0)
            ones16 = const.tile([1, 128], F16)
            nc.vector.memset(ones16, 1.0)
            z128 = const.tile([1, 128], F16)
            nc.vector.memset(z128, 0.0)
            z512 = const.tile([1, 512], F16)
            nc.vector.memset(z512, 0.0)

            # preload the exp table set while input DMAs run
            scratch1 = const.tile([1, 4], F32)
            nc.scalar.activation(scratch1, ones1[0:1, 0:4], EXP)

            # xTd/wadst first: they gate the q_bc chain that gates the grid
            xTd = const.tile([F, ID], F16)
            nc.sync.dma_start(out=xTd, in_=d_xTd)
            wadst = const.tile([F, H], F16)
            nc.sync.dma_start(out=wadst, in_=d_wadst)
            wcat = const.tile([F, HC + 8], F16)
            nc.sync.dma_start(out=wcat, in_=d_wcat)
            xT = const.tile([F, N], F16)
            for c in range(2):
                nc.sync.dma_start(out=xT[:, c * 1024:(c + 1) * 1024],
                                  in_=d_xT[:, c * 1024:(c + 1) * 1024])
            bias_sb = const.tile([1, HC + 8], F16)
            nc.sync.dma_start(out=bias_sb, in_=d_bias)
            # one-hot selector rows: E4[h] broadcasts qrow4 row h via K=4 MM
            e4 = const.tile([4, H, 128], F16)
            nc.sync.dma_start(out=e4, in_=d_e4)

            # persistent per-core tensors
            xh1 = const.tile([128, NT, H, 65], F16)     # [xh+bias | 1] per (t,h)
            xh2b = const.tile([128, NT, H, 65], F16)    # E2-scaled xh1 (ACT tiles)
            expv = const.tile([128, NT, 8], F32)        # exp(.2 a_src) | exp(a_src)
            nexpv = const.tile([128, NT, 4], F32)       # -exp(a_src) (ACT bias)
            q_bc = const.tile([128, H, ID], F16)        # Q' broadcast per head
            qrow4 = const.tile([4, 2, 512], F16)        # exp(-0.8 a_dst), 4 rows

            tacts = {}
            # ---------------- phase A: projections ----------------
            with tc.tile_pool(name="psA", bufs=2, space="PSUM") as psA, \
                 tc.tile_pool(name="psD", bufs=2, space="PSUM") as psDp, \
                 tc.tile_pool(name="psQ", bufs=2, space="PSUM") as psQp, \
                 tc.tile_pool(name="psB", bufs=2, space="PSUM") as psBp:
                # ones column of every xh1 block (cols 0:64 written below);
                # DVE memset keeps GpSimd completely unused (its sequencer
                # otherwise adds library-reload + end-barrier housekeeping)
                nc.vector.memset(xh1[:, :, :, 64:65], 1.0)
                sc_a = nc.enter_named_scope("phA", False)
                # --- q_bc prefix first: it gates the grid loop.  a_dst
                # projection (all 4 heads in one MM) -> exp -> K=4 one-hot
                # stationary matmul broadcasts row h to all 128 partitions.
                # head 0 is copied PSUM->SBUF on DVE (idle at startup) so the
                # grid pipeline starts asap; the rest go through ACT.
                for k in range(2):
                    psd = psDp.tile([4, 512], F32)
                    nc.tensor.matmul(psd, wadst,
                                     xTd[:, k * 512:(k + 1) * 512],
                                     start=True, stop=True)
                    nc.scalar.activation(qrow4[:, k, :], psd, EXP)
                for h in range(H):
                    for k in range(2):
                        psq = psQp.tile([128, 512], F32)
                        nc.tensor.matmul(psq, e4[:, h, :], qrow4[:, k, :],
                                         start=True, stop=True)
                        nc.scalar.activation(
                            q_bc[:, h, k * 512:(k + 1) * 512], psq, CPY)
                # projection tiles; grid tile t can start once tile t is done
                # (second K=1 matmul accumulates the bias row into the psum,
                # so xh1 = xh + bias with no DVE work — num+bias*den trick)
                for t in range(NT):
                    ps = psA.tile([128, HC + 8], F32)
                    nc.tensor.matmul(ps, xT[:, t * 128:(t + 1) * 128], wcat,
                                     start=True, stop=False)
                    nc.tensor.matmul(ps, ones16, bias_sb,
                                     start=False, stop=True)
                    # exp of the 8 pre-scaled projection cols
                    nc.scalar.activation(expv[:, t, :], ps[:, HC:HC + 8], EXP)
                    # xh+bias into the 65-column head blocks
                    nc.scalar.activation(xh1[:, t, :, 0:64], ps[:, 0:HC], CPY)
                    if t in acts:
                        # negated E2 for the ACT relu bias
                        nc.vector.tensor_scalar(
                            out=nexpv[:, t, :], in0=expv[:, t, 4:8],
                            scalar1=-1.0, scalar2=None,
                            op0=mybir.AluOpType.mult)
                        # E2-scaled stationary for the separable branch
                        for h in range(H):
                            nc.vector.tensor_scalar(
                                out=xh2b[:, t, h, :], in0=xh1[:, t, h, :],
                                scalar1=expv[:, t, 4 + h:5 + h], scalar2=None,
                                op0=mybir.AluOpType.mult)
                        # T = relu(E1_j * Q'_i - E2_j) emitted HERE so the
                        # Scalar engine computes it long before phB's TT
                        # needs it (DVE is strict FIFO — a late T would
                        # head-of-line-block every later DVE op)
                        ta = const.tile([128, H, ID], F16, name=f"tact{t}")
                        for h in range(H):
                            nc.scalar.activation(
                                ta[:, h, :], q_bc[:, h, :], RELU,
                                bias=nexpv[:, t, h:h + 1],
                                scale=expv[:, t, h:h + 1])
                        tacts[t] = ta
                nc.leave_named_scope("phA", sc_a[0], False)

            # ---------------- phase B: grid + matmul accumulate ----------------
            with tc.tile_pool(name="acc", bufs=1, space="PSUM") as accp:
                acc = {}
                for ib in range(NIB):
                    acc[ib] = accp.tile([128, H, 128], F32, tag=f"acc{ib}",
                                        name=f"acc{ib}")

                sc_b = nc.enter_named_scope("phB", False)
                # one whole-bank zeroing matmul per acc bank: carries the only
                # start=True, so per-head accumulate groups sharing a bank
                # can't clear each other's has_written bits
                for ib in range(NIB):
                    accf = bass.AP(
                        tensor=acc[ib].tensor, offset=acc[ib].offset,
                        ap=[acc[ib].ap[0], [1, H * 128]])
                    nc.tensor.matmul(accf, z128, z512, start=True, stop=False)
                with tc.tile_pool(name="adj", bufs=ABUFS) as adjp, \
                     tc.tile_pool(name="grid", bufs=4) as gridp:
                    for rep in range(reps):
                        for t in range(NT):
                            adjt = adjp.tile([128, ID], F16)
                            nc.sync.dma_start(out=adjt, in_=d_adjT[t])
                            edge = (t in (0, NT - 1)
                                    and t not in acts and t not in gpss)
                            if t in acts:
                                t_all = tacts[t]   # precomputed in phA on ACT
                            elif not edge:
                                t_all = gridp.tile([128, H, ID], F16, tag="T",
                                                   bufs=TBUFS)
                                for h in range(H):
                                    # T = max(Q'_i * E1_j, E2_j) on DVE
                                    nc.vector.tensor_scalar(
                                        out=t_all[:, h, :], in0=q_bc[:, h, :],
                                        scalar1=expv[:, t, h:h + 1],
                                        scalar2=expv[:, t, 4 + h:5 + h],
                                        op0=mybir.AluOpType.mult,
                                        op1=mybir.AluOpType.max)
                            g = gridp.tile([128, H, ID], F16, tag="G", bufs=GBUFS)
                            if edge:
                                # first/last tile: chunk-split T+TT so the
                                # first MMs start (t=0) / the epilogue's MMs
                                # finish (t=15) half a grid earlier
                                t_all = gridp.tile([128, H, ID], F16, tag="T",
                                                   bufs=TBUFS)
                                for k2 in range(2):
                                    sl = slice(k2 * 512, (k2 + 1) * 512)
                                    for h in range(H):
                                        nc.vector.tensor_scalar(
                                            out=t_all[:, h, sl],
                                            in0=q_bc[:, h, sl],
                                            scalar1=expv[:, t, h:h + 1],
                                            scalar2=expv[:, t, 4 + h:5 + h],
                                            op0=mybir.AluOpType.mult,
                                            op1=mybir.AluOpType.max)
                                    tv = bass.AP(
                                        tensor=t_all.tensor,
                                        offset=t_all.offset + k2 * 512,
                                        ap=[t_all.ap[0], [ID, H], [1, 512]])
                                    gv = bass.AP(
                                        tensor=g.tensor,
                                        offset=g.offset + k2 * 512,
                                        ap=[g.ap[0], [ID, H], [1, 512]])
                                    adj_rep4k = bass.AP(
                                        tensor=adjt.tensor,
                                        offset=adjt.offset + k2 * 512,
                                        ap=[adjt.ap[0], [0, H], [1, 512]])
                                    nc.vector.tensor_tensor(
                                        out=gv, in0=tv, in1=adj_rep4k,
                                        op=mybir.AluOpType.mult)
                            elif t in gpss:
                                # gpsimd path: per-head TT (natural in1 AP)
                                for h in range(H):
                                    nc.gpsimd.tensor_tensor(
                                        out=g[:, h, :], in0=t_all[:, h, :],
                                        in1=adjt, op=mybir.AluOpType.mult)
                            else:
                                adj_rep4 = bass.AP(
                                    tensor=adjt.tensor, offset=adjt.offset,
                                    ap=[adjt.ap[0], [0, H]] + list(adjt.ap[1:]))
                                nc.vector.tensor_tensor(
                                    out=g, in0=t_all, in1=adj_rep4,
                                    op=mybir.AluOpType.mult)
                            last = (rep == reps - 1 and t == NT - 1)
                            for ib in range(NIB):
                                for h in range(H):
                                    # acc[ib][i, h, :] += G_blk^T @ [xh|1]
                                    nc.tensor.matmul(
                                        acc[ib][:, h, 0:65],
                                        g[:, h, ib * 128:(ib + 1) * 128],
                                        xh1[:, t, h, :],
                                        start=False, stop=last)
                                if t in acts:
                                    # separable E2 branch: shared adj block
                                    # stationary, all 4 heads' E2-xh moving
                                    nc.tensor.matmul(
                                        acc[ib][:, :, 0:65],
                                        adjt[:, ib * 128:(ib + 1) * 128],
                                        xh2b[:, t, :, :],
                                        start=False, stop=False)

                nc.leave_named_scope("phB", sc_b[0], False)
                sc_c = nc.enter_named_scope("phC", False)
                # ------------- phase C: divide + out (no transposes) -------------
                with tc.tile_pool(name="ep_sm", bufs=8) as epsm, \
                     tc.tile_pool(name="outp", bufs=2) as outp:
                    # 4 dest blocks share one osb group so every partition
                    # DMAs 4KB contiguous (host un-permutes the row order)
                    for g in range(NIB // 4):
                        osb = outp.tile([128, 4, HC], F32, tag="osb",
                                        name="osb")
                        for kb in range(4):
                            ib = g * 4 + kb
                            rec = epsm.tile([128, H, 1], F32)
                            nc.vector.reciprocal(rec, acc[ib][:, :, 64:65])
                            for h in range(H):
                                if h % 2 == 0:
                                    nc.vector.tensor_scalar(
                                        out=osb[:, kb, h * 64:(h + 1) * 64],
                                        in0=acc[ib][:, h, 0:64],
                                        scalar1=rec[:, h, :], scalar2=None,
                                        op0=mybir.AluOpType.mult)
                                else:
                                    nc.scalar.activation(
                                        osb[:, kb, h * 64:(h + 1) * 64],
                                        acc[ib][:, h, 0:64], CPY,
                                        scale=rec[:, h, :])
                        blk = d_out[g * 512:(g + 1) * 512, :]
                        out_ap = bass.AP(
                            tensor=blk.tensor, offset=blk.offset,
                            ap=[[4 * HC, 128], [HC, 4], [1, HC]])
                        nc.sync.dma_start(out=out_ap, in_=osb)
                nc.leave_named_scope("phC", sc_c[0], False)

    nc.compile()
    return nc


def _get_nc(reps: int = 1):
    if reps not in _NC_CACHE:
        _NC_CACHE[reps] = build_nc(reps)
    return _NC_CACHE[reps]


def make_in_maps(x, adj, W, att_src, att_dst, bias):
    x = np.asarray(x, dtype=np.float32)
    adj = np.asarray(adj, dtype=np.float32)
    W = np.asarray(W, dtype=np.float32)
    att_src = np.asarray(att_src, dtype=np.float32)
    att_dst = np.asarray(att_dst, dtype=np.float32)
    bias = np.asarray(bias, dtype=np.float32)

    # weight prep: fold per-head attention dots into projection columns
    wa_src = np.stack([W[:, h * C:(h + 1) * C] @ att_src[h] for h in range(H)], 1)
    wa_dst = np.stack([W[:, h * C:(h + 1) * C] @ att_dst[h] for h in range(H)], 1)
    wcat = np.concatenate([W, 0.2 * wa_src, wa_src], axis=1)
    wcat = np.ascontiguousarray(wcat, dtype=np.float16)          # [F, 264]
    wadst = np.ascontiguousarray(-0.8 * wa_dst, dtype=np.float16)  # [F, 4]

    adjl = adj.copy()
    idx = np.arange(N)
    adjl[:, idx, idx] = 1.0

    # one-hot broadcast selector: e4sel[p, h*128+c] = (p == h)
    e4sel = np.zeros((4, H, 128), np.float16)
    for h in range(H):
        e4sel[h, h, :] = 1.0
    e4sel = e4sel.reshape(4, H * 128)

    in_maps = []
    for c in range(N_CORES):
        b, half = c // 2, c % 2
        xT = np.ascontiguousarray(x[b].T, dtype=np.float16)
        xTd = np.ascontiguousarray(x[b, half * ID:(half + 1) * ID, :].T,
                                   dtype=np.float16)
        adjT = np.ascontiguousarray(
            adjl[b].T[:, half * ID:(half + 1) * ID]).astype(np.float16)
        in_maps.append({
            "xT": xT,
            "xTd": xTd,
            "adjT": adjT.reshape(NT, 128, ID),
            "Wcat": wcat,
            "Wadst": wadst,
            "biasv": np.concatenate([bias, np.zeros(8, np.float32)]).reshape(
                1, HC + 8).astype(np.float16),
            "e4sel": e4sel,
        })
    return in_maps


def assemble(results):
    # un-permute the 4-blocks-per-DMA row grouping: DRAM row
    # (ib//4)*512 + 4*p + ib%4 holds original row ib*128 + p
    i = np.arange(ID)
    ib, p = i // 128, i % 128
    r = (ib // 4) * 512 + 4 * p + (ib % 4)
    out = np.empty((B, N, HC), dtype=np.float32)
    for c in range(N_CORES):
        b, half = c // 2, c % 2
        out[b, half * ID:(half + 1) * ID, :] = results[c]["out"][r]
    return out


def kernel(x, adj, W, att_src, att_dst, bias):
    nc = _get_nc(1)
    in_maps = make_in_maps(x, adj, W, att_src, att_dst, bias)
    res = run_bass_kernel_spmd(nc, in_maps, list(range(N_CORES)))
    return assemble(res.results)


# revision 50
# speedup vs baseline: 1.0190x; 1.0053x over previous
"""DenseGATConv Bass/Tile kernel for Trainium2, SPMD over 8 NeuronCores.

Problem (B=4, N=2048, F=128, H=4, C=64):
  xh = (x @ W).reshape(B,N,H,C)
  a_src[b,j,h] = xh . att_src ; a_dst[b,i,h] = xh . att_dst
  s = a_src[j] + a_dst[i];  alpha = softmax_j(mask(adj+I, leaky_relu(s, 0.2)))
  out[b,i] = concat_h(sum_j alpha * xh[b,j,h,:]) + bias

Algebra (no exp over the N*N*H grid, no softmax normalizer subtraction):
  exp(lrelu(s)) / exp(a_dst_i) = max(E1_j * Q'_i, E2_j),
      E1 = exp(0.2 a_src), E2 = exp(a_src), Q' = exp(-0.8 a_dst)
  Masked grid weight  G[j,i] = adjT[j,i] * max(E1_j Q'_i, E2_j).

Work split per (j-tile, head) — all on the DVE by default:
  T = tensor_scalar(Q'_bcast; mult E1, max E2)   (4x mode, 459 ns)
  G = tensor_tensor(T, adjT rep-AP, all 4 heads) (2x mode, 2.2 us)
(ACTN/GPSN env knobs exist to shift T to the Scalar engine - with the
separable E2 branch restored by an extra shared-adj-stationary matmul -
or the TT to GpSimd; both measured net-negative on HW: ACT is 3x slower
per element and strict-FIFO ordering stalls the pipeline, and GpSimd
SBUF-port contention slows concurrent DVE ops ~2.5x.  Default 0/0.)

Accumulation (flipped orientation — no epilogue transposes):
  For each (tile t, i-block ib of 128, head h):
      acc[ib][i, h, c|den] += G_block[j, i]^T @ xh1[j, (c|1)]
  i.e. the 128x128 grid block is the *stationary* operand (FWL-eligible
  fp16 128-col load) and xh1 streams 65 cols.  PSUM acc2[ib] is a full
  bank [128, 4, 128(pad)]; col 64 of each head slot is the softmax
  denominator.  Bias is pre-folded into xh1 (num+bias*den trick), so the
  epilogue is just reciprocal + per-partition tensor_scalar divide + DMA.

Q'_i is broadcast to all 128 partitions with a K=1 ones-stationary
matmul (PSUM bounce) instead of a DRAM roundtrip.

Sharding: core = b*2 + ihalf; each core owns 1024 destination rows of one
batch and reads that batch's full source side (adj slice pre-transposed,
self-loops added, fp16-cast on host; weights pre-folded with the per-head
attention vectors; x / W / projections run in fp16).
"""

import numpy as np

import concourse.bacc as bacc
import concourse.bass as bass
import concourse.tile as tile
from concourse import mybir
from concourse.bass_utils import run_bass_kernel_spmd
from concourse.masks import make_identity

B, N, F = 4, 2048, 128
H, C = 4, 64
HC = H * C
NEG_SLOPE = 0.2
import os
TBUFS = int(os.environ.get('TBUFS', 5))
GBUFS = int(os.environ.get('GBUFS', 6))
ABUFS = int(os.environ.get('ABUFS', 4))
ACTN = int(os.environ.get('ACTN', 0))   # of 16 j-tiles use the ACT path
GPSN = int(os.environ.get('GPSN', 0))   # of 16 j-tiles run their TT on GpSimd
N_CORES = 8
ID = N // 2          # dest rows per core
NT = N // 128        # 16 source tiles
NIB = ID // 128      # 8 dest 128-blocks
F32 = mybir.dt.float32
F16 = mybir.dt.float16

_NC_CACHE = {}


def tile_assignment(actn: int, gpsn: int) -> tuple:
    """Spread actn ACT-path tiles and gpsn gpsimd-TT tiles over the 16
    j-tiles (disjoint sets).  ACT tiles start at t=4 so the Scalar engine
    has drained its startup queue by the time their T grids are needed."""
    actn = max(0, min(actn, NT))
    gpsn = max(0, min(gpsn, NT - actn))
    order = [4, 6, 8, 10, 12, 14, 3, 9, 13, 5, 11, 7, 2, 1, 0, 15]
    acts = set(order[:actn])
    gpss = set(order[actn:actn + gpsn])
    return acts, gpss


def build_nc(reps: int = 1):
    nc = bacc.Bacc("TRN2", target_bir_lowering=False, debug=False, num_devices=1)

    d_xT = nc.dram_tensor("xT", [F, N], F16, kind="ExternalInput").ap()
    d_xTd = nc.dram_tensor("xTd", [F, ID], F16, kind="ExternalInput").ap()
    d_adjT = nc.dram_tensor("adjT", [NT, 128, ID], F16, kind="ExternalInput").ap()
    d_wcat = nc.dram_tensor("Wcat", [F, HC + 8], F16, kind="ExternalInput").ap()
    d_wadst = nc.dram_tensor("Wadst", [F, H], F16, kind="ExternalInput").ap()
    d_bias = nc.dram_tensor("biasv", [1, HC + 8], F16, kind="ExternalInput").ap()
    d_e4 = nc.dram_tensor("e4sel", [4, H * 128], F16, kind="ExternalInput").ap()
    d_out = nc.dram_tensor("out", [ID, HC], F32, kind="ExternalOutput").ap()

    EXP = mybir.ActivationFunctionType.Exp
    CPY = mybir.ActivationFunctionType.Copy
    RELU = mybir.ActivationFunctionType.Relu
    acts, gpss = tile_assignment(ACTN, GPSN)

    with tile.TileContext(nc) as tc:
        with tc.tile_pool(name="const", bufs=1) as const:
            ones1 = const.tile([1, 128], F32)
            nc.vector.memset(ones1, 1.0)
            ones16 = const.tile([1, 128], F16)
            nc.vector.memset(ones16, 1.0)
            z128 = const.tile([1, 128], F16)
            nc.vector.memset(z128, 0.0)
            z512 = const.tile([1, 512], F16)
            nc.vector.memset(z512, 0.0)

            # preload the exp table set while input DMAs run
            scratch1 = const.tile([1, 4], F32)
            nc.scalar.activation(scratch1, ones1[0:1, 0:4], EXP)

            # xTd/wadst first: they gate the q_bc chain that gates the grid
            xTd = const.tile([F, ID], F16)
            nc.sync.dma_start(out=xTd, in_=d_xTd)
            wadst = const.tile([F, H], F16)
            nc.sync.dma_start(out=wadst, in_=d_wadst)
            wcat = const.tile([F, HC + 8], F16)
            nc.sync.dma_start(out=wcat, in_=d_wcat)
            xT = const.tile([F, N], F16)
            for c in range(2):
                nc.sync.dma_start(out=xT[:, c * 1024:(c + 1) * 1024],
                                  in_=d_xT[:, c * 1024:(c + 1) * 1024])
            bias_sb = const.tile([1, HC + 8], F16)
            nc.sync.dma_start(out=bias_sb, in_=d_bias)
            # one-hot selector rows: E4[h] broadcasts qrow4 row h via K=4 MM
            e4 = const.tile([4, H, 128], F16)
            nc.sync.dma_start(out=e4, in_=d_e4)

            # persistent per-core tensors
            xh1 = const.tile([128, NT, H, 65], F16)     # [xh+bias | 1] per (t,h)
            xh2b = const.tile([128, NT, H, 65], F16)    # E2-scaled xh1 (ACT tiles)
            expv = const.tile([128, NT, 8], F32)        # exp(.2 a_src) | exp(a_src)
            nexpv = const.tile([128, NT, 4], F32)       # -exp(a_src) (ACT bias)
            q_bc = const.tile([128, H, ID], F16)        # Q' broadcast per head
            qrow4 = const.tile([4, 2, 512], F16)        # exp(-0.8 a_dst), 4 rows

            tacts = {}
            # ---------------- phase A: projections ----------------
            with tc.tile_pool(name="psA", bufs=2, space="PSUM") as psA, \
                 tc.tile_pool(name="psD", bufs=2, space="PSUM") as psDp, \
                 tc.tile_pool(name="psQ", bufs=2, space="PSUM") as psQp, \
                 tc.tile_pool(name="psB", bufs=2, space="PSUM") as psBp:
                # ones column of every xh1 block (cols 0:64 written below);
                # DVE memset keeps GpSimd completely unused (its sequencer
                # otherwise adds library-reload + end-barrier housekeeping)
                nc.vector.memset(xh1[:, :, :, 64:65], 1.0)
                sc_a = nc.enter_named_scope("phA", False)
                # --- q_bc prefix first: it gates the grid loop.  a_dst
                # projection (all 4 heads in one MM) -> exp -> K=4 one-hot
                # stationary matmul broadcasts row h to all 128 partitions.
                # head 0 is copied PSUM->SBUF on DVE (idle at startup) so the
                # grid pipeline starts asap; the rest go through ACT.
                for k in range(2):
                    psd = psDp.tile([4, 512], F32)
                    nc.tensor.matmul(psd, wadst,
                                     xTd[:, k * 512:(k + 1) * 512],
                                     start=True, stop=True)
                    nc.scalar.activation(qrow4[:, k, :], psd, EXP)
                for h in range(H):
                    for k in range(2):
                        psq = psQp.tile([128, 512], F32)
                        nc.tensor.matmul(psq, e4[:, h, :], qrow4[:, k, :],
                                         start=True, stop=True)
                        nc.scalar.activation(
                            q_bc[:, h, k * 512:(k + 1) * 512], psq, CPY)
                # projection tiles; grid tile t can start once tile t is done
                # (second K=1 matmul accumulates the bias row into the psum,
                # so xh1 = xh + bias with no DVE work — num+bias*den trick)
                for t in range(NT):
                    ps = psA.tile([128, HC + 8], F32)
                    nc.tensor.matmul(ps, xT[:, t * 128:(t + 1) * 128], wcat,
                                     start=True, stop=False)
                    nc.tensor.matmul(ps, ones16, bias_sb,
                                     start=False, stop=True)
                    # exp of the 8 pre-scaled projection cols
                    nc.scalar.activation(expv[:, t, :], ps[:, HC:HC + 8], EXP)
                    # xh+bias into the 65-column head blocks
                    nc.scalar.activation(xh1[:, t, :, 0:64], ps[:, 0:HC], CPY)
                    if t in acts:
                        # negated E2 for the ACT relu bias
                        nc.vector.tensor_scalar(
                            out=nexpv[:, t, :], in0=expv[:, t, 4:8],
                            scalar1=-1.0, scalar2=None,
                            op0=mybir.AluOpType.mult)
                        # E2-scaled stationary for the separable branch
                        for h in range(H):
                            nc.vector.tensor_scalar(
                                out=xh2b[:, t, h, :], in0=xh1[:, t, h, :],
                                scalar1=expv[:, t, 4 + h:5 + h], scalar2=None,
                                op0=mybir.AluOpType.mult)
                        # T = relu(E1_j * Q'_i - E2_j) emitted HERE so the
                        # Scalar engine computes it long before phB's TT
                        # needs it (DVE is strict FIFO — a late T would
                        # head-of-line-block every later DVE op)
                        ta = const.tile([128, H, ID], F16, name=f"tact{t}")
                        for h in range(H):
                            nc.scalar.activation(
                                ta[:, h, :], q_bc[:, h, :], RELU,
                                bias=nexpv[:, t, h:h + 1],
                                scale=expv[:, t, h:h + 1])
                        tacts[t] = ta
                nc.leave_named_scope("phA", sc_a[0], False)

            # ---------------- phase B: grid + matmul accumulate ----------------
            with tc.tile_pool(name="acc", bufs=1, space="PSUM") as accp:
                acc = {}
                for ib in range(NIB):
                    acc[ib] = accp.tile([128, H, 128], F32, tag=f"acc{ib}",
                                        name=f"acc{ib}")

                sc_b = nc.enter_named_scope("phB", False)
                # one whole-bank zeroing matmul per acc bank: carries the only
                # start=True, so per-head accumulate groups sharing a bank
                # can't clear each other's has_written bits
                for ib in range(NIB):
                    accf = bass.AP(
                        tensor=acc[ib].tensor, offset=acc[ib].offset,
                        ap=[acc[ib].ap[0], [1, H * 128]])
                    nc.tensor.matmul(accf, z128, z512, start=True, stop=False)
                with tc.tile_pool(name="adj", bufs=ABUFS) as adjp, \
                     tc.tile_pool(name="grid", bufs=4) as gridp:
                    for rep in range(reps):
                        for t in range(NT):
                            adjt = adjp.tile([128, ID], F16)
                            nc.sync.dma_start(out=adjt, in_=d_adjT[t])
                            edge = (t in (0, NT - 1)
                                    and t not in acts and t not in gpss)
                            if t in acts:
                                t_all = tacts[t]   # precomputed in phA on ACT
                            elif not edge:
                                t_all = gridp.tile([128, H, ID], F16, tag="T",
                                                   bufs=TBUFS)
                                for h in range(H):
                                    # T = max(Q'_i * E1_j, E2_j) on DVE
                                    nc.vector.tensor_scalar(
                                        out=t_all[:, h, :], in0=q_bc[:, h, :],
                                        scalar1=expv[:, t, h:h + 1],
                                        scalar2=expv[:, t, 4 + h:5 + h],
                                        op0=mybir.AluOpType.mult,
                                        op1=mybir.AluOpType.max)
                            g = gridp.tile([128, H, ID], F16, tag="G", bufs=GBUFS)
                            if edge:
                                # first/last tile: chunk-split T+TT so the
                                # first MMs start (t=0) / the epilogue's MMs
                                # finish (t=15) half a grid earlier
                                t_all = gridp.tile([128, H, ID], F16, tag="T",
                                                   bufs=TBUFS)
                                for k2 in range(2):
                                    sl = slice(k2 * 512, (k2 + 1) * 512)
                                    for h in range(H):
                                        nc.vector.tensor_scalar(
                                            out=t_all[:, h, sl],
                                            in0=q_bc[:, h, sl],
                                            scalar1=expv[:, t, h:h + 1],
                                            scalar2=expv[:, t, 4 + h:5 + h],
                                            op0=mybir.AluOpType.mult,
                                            op1=mybir.AluOpType.max)
                                    tv = bass.AP(
                                        tensor=t_all.tensor,
                                        offset=t_all.offset + k2 * 512,
                                        ap=[t_all.ap[0], [ID, H], [1, 512]])
                                    gv = bass.AP(
                                        tensor=g.tensor,
                                        offset=g.offset + k2 * 512,
                                        ap=[g.ap[0], [ID, H], [1, 512]])
                                    adj_rep4k = bass.AP(
                                        tensor=adjt.tensor,
                                        offset=adjt.offset + k2 * 512,
                                        ap=[adjt.ap[0], [0, H], [1, 512]])
                                    nc.vector.tensor_tensor(
                                        out=gv, in0=tv, in1=adj_rep4k,
                                        op=mybir.AluOpType.mult)
                            elif t in gpss:
                                # gpsimd path: per-head TT (natural in1 AP)
                                for h in range(H):
                                    nc.gpsimd.tensor_tensor(
                                        out=g[:, h, :], in0=t_all[:, h, :],
                                        in1=adjt, op=mybir.AluOpType.mult)
                            else:
                                adj_rep4 = bass.AP(
                                    tensor=adjt.tensor, offset=adjt.offset,
                                    ap=[adjt.ap[0], [0, H]] + list(adjt.ap[1:]))
                                nc.vector.tensor_tensor(
                                    out=g, in0=t_all, in1=adj_rep4,
                                    op=mybir.AluOpType.mult)
                            last = (rep == reps - 1 and t == NT - 1)
                            for ib in range(NIB):
                                for h in range(H):
                                    # acc[ib][i, h, :] += G_blk^T @ [xh|1]
                                    nc.tensor.matmul(
                                        acc[ib][:, h, 0:65],
                                        g[:, h, ib * 128:(ib + 1) * 128],
                                        xh1[:, t, h, :],
                                        start=False, stop=last)
                                if t in acts:
                                    # separable E2 branch: shared adj block
                                    # stationary, all 4 heads' E2-xh moving
                                    nc.tensor.matmul(
                                        acc[ib][:, :, 0:65],
                                        adjt[:, ib * 128:(ib + 1) * 128],
                                        xh2b[:, t, :, :],
                                        start=False, stop=False)

                nc.leave_named_scope("phB", sc_b[0], False)
                sc_c = nc.enter_named_scope("phC", False)
                # ------------- phase C: divide + out (no transposes) -------------
                with tc.tile_pool(name="ep_sm", bufs=8) as epsm, \
                     tc.tile_pool(name="outp", bufs=4) as outp:
                    # 2 dest blocks share one osb group so every partition
                    # DMAs 2KB contiguous (host un-permutes the row order);
                    # smaller groups let the final transfer start earlier
                    GRP = 2
                    for g in range(NIB // GRP):
                        osb = outp.tile([128, GRP, HC], F32, tag="osb",
                                        name="osb")
                        for kb in range(GRP):
                            ib = g * GRP + kb
                            rec = epsm.tile([128, H, 1], F32)
                            nc.vector.reciprocal(rec, acc[ib][:, :, 64:65])
                            for h in range(H):
                                if h % 2 == 0:
                                    nc.vector.tensor_scalar(
                                        out=osb[:, kb, h * 64:(h + 1) * 64],
                                        in0=acc[ib][:, h, 0:64],
                                        scalar1=rec[:, h, :], scalar2=None,
                                        op0=mybir.AluOpType.mult)
                                else:
                                    nc.scalar.activation(
                                        osb[:, kb, h * 64:(h + 1) * 64],
                                        acc[ib][:, h, 0:64], CPY,
                                        scale=rec[:, h, :])
                        blk = d_out[g * GRP * 128:(g + 1) * GRP * 128, :]
                        out_ap = bass.AP(
                            tensor=blk.tensor, offset=blk.offset,
                            ap=[[GRP * HC, 128], [HC, GRP], [1, HC]])
                        nc.sync.dma_start(out=out_ap, in_=osb)
                nc.leave_named_scope("phC", sc_c[0], False)

    nc.compile()
    return nc


def _get_nc(reps: int = 1):
    if reps not in _NC_CACHE:
        _NC_CACHE[reps] = build_nc(reps)
    return _NC_CACHE[reps]


def make_in_maps(x, adj, W, att_src, att_dst, bias):
    x = np.asarray(x, dtype=np.float32)
    adj = np.asarray(adj, dtype=np.float32)
    W = np.asarray(W, dtype=np.float32)
    att_src = np.asarray(att_src, dtype=np.float32)
    att_dst = np.asarray(att_dst, dtype=np.float32)
    bias = np.asarray(bias, dtype=np.float32)

    # weight prep: fold per-head attention dots into projection columns
    wa_src = np.stack([W[:, h * C:(h + 1) * C] @ att_src[h] for h in range(H)], 1)
    wa_dst = np.stack([W[:, h * C:(h + 1) * C] @ att_dst[h] for h in range(H)], 1)
    wcat = np.concatenate([W, 0.2 * wa_src, wa_src], axis=1)
    wcat = np.ascontiguousarray(wcat, dtype=np.float16)          # [F, 264]
    wadst = np.ascontiguousarray(-0.8 * wa_dst, dtype=np.float16)  # [F, 4]

    adjl = adj.copy()
    idx = np.arange(N)
    adjl[:, idx, idx] = 1.0

    # one-hot broadcast selector: e4sel[p, h*128+c] = (p == h)
    e4sel = np.zeros((4, H, 128), np.float16)
    for h in range(H):
        e4sel[h, h, :] = 1.0
    e4sel = e4sel.reshape(4, H * 128)

    in_maps = []
    for c in range(N_CORES):
        b, half = c // 2, c % 2
        xT = np.ascontiguousarray(x[b].T, dtype=np.float16)
        xTd = np.ascontiguousarray(x[b, half * ID:(half + 1) * ID, :].T,
                                   dtype=np.float16)
        adjT = np.ascontiguousarray(
            adjl[b].T[:, half * ID:(half + 1) * ID]).astype(np.float16)
        in_maps.append({
            "xT": xT,
            "xTd": xTd,
            "adjT": adjT.reshape(NT, 128, ID),
            "Wcat": wcat,
            "Wadst": wadst,
            "biasv": np.concatenate([bias, np.zeros(8, np.float32)]).reshape(
                1, HC + 8).astype(np.float16),
            "e4sel": e4sel,
        })
    return in_maps


def assemble(results):
    # un-permute the 2-blocks-per-DMA row grouping: DRAM row
    # (ib//2)*256 + 2*p + ib%2 holds original row ib*128 + p
    i = np.arange(ID)
    ib, p = i // 128, i % 128
    r = (ib // 2) * 256 + 2 * p + (ib % 2)
    out = np.empty((B, N, HC), dtype=np.float32)
    for c in range(N_CORES):
        b, half = c // 2, c % 2
        out[b, half * ID:(half + 1) * ID, :] = results[c]["out"][r]
    return out


def kernel(x, adj, W, att_src, att_dst, bias):
    nc = _get_nc(1)
    in_maps = make_in_maps(x, adj, W, att_src, att_dst, bias)
    res = run_bass_kernel_spmd(nc, in_maps, list(range(N_CORES)))
    return assemble(res.results)
